# revision 3
# baseline (speedup 1.0000x reference)
"""Causal multi-head decoder attention on 8 Trainium2 NeuronCores.

Problem shapes (hardcoded): x [B=2, S=2048, D=1024], 16 heads x d_head=64.
Sharding: core c -> (batch b = c//4, head-group hg = c%4 covering 4 heads).
Attention is fully head-local; each core computes the partial output
projection for its 4 heads, and the host sums the 4 partials per batch
(the "output projection all-reduce") during unshard.

On-device layout strategy (per core):
  - host provides xT = x[b].T  [1024, 2048] so Q/K projections directly
    produce qT/kT [64, S] (head-dim on partitions) with no transposes.
  - Q/K projections produce head PAIRS stacked in partition halves
    (qkT[:, r, :]: rows 0:64 = head 2r', 64:128 = head 2r'+1), so the
    score matmuls for a head pair run as two concurrent 64-row-group
    matmuls on the PE array (tile_position) with NO duplication DMAs.
  - V is computed in [S, 64] orientation (x-chunk stationary) and stored
    interleaved with a ones-column per head: vaug [128, 16sc, 4h*65].
    The ones-column makes the attn@V matmul also produce the softmax
    denominator row (zaug [65, 512] = 64 z rows + 1 denom row).
  - scoresT [ki, qi] = kT-chunk.T @ qT-tile (contraction over d_head=64).
    exp() on the scalar engine; causal mask applied multiplicatively on
    the diagonal 128-blocks only; for diagonal chunks the score matmul /
    exp / attn@V are all narrowed to the un-masked qi column range.
  - kc ascending accumulation for zaug; the kc loop is software-pipelined
    depth 2 (scores(kc+1) issued before attn@V(kc)) so the PE never waits
    for the exp; Q/K/V projection psum-groups of the NEXT s-tile are
    interleaved into the attention loop as fillers to keep the PE fed
    during scalar-bound windows.
  - 1/sqrt(d_head) folded into the K weights host-side.
  - normalize z via vector reciprocal of the denom row, broadcast across
    partitions with a tiny K=1 f16 matmul against a ones row.
  - output projection: O stacked per head-pair so contraction is 128-wide.
  - input DMA spread over 5 engine queues, first s-tile chunks first.
"""

import os as _os

import numpy as np

import concourse.bass as bass
import concourse.tile as tile
from concourse import mybir
from concourse.bass_utils import run_bass_kernel_spmd

F32 = mybir.dt.float32
F16 = mybir.dt.float16

B, S, D, NH, DH = 2, 2048, 1024, 16, 64
HL = 4            # heads per core
DC = D // 128     # 8 d-chunks
NQT = S // 512    # 4 qi tiles
NSC = S // 128    # 16 128-token chunks
IGNORE = -100000.0

# ---------------------------------------------------------------------------
# Workaround for this walrus build's per-instruction sync-wait budget of one
# ("Too many sync wait commands"): after Tile scheduling, move excess waits
# from any instruction onto same-engine NoOps inserted just before it.
MAX_WAITS = 1


def _split_sync_waits(nc, max_waits=MAX_WAITS):
    k = 0
    for fn in nc.m.functions:
        for bb in fn.blocks:
            insts = bb.instructions
            i = 0
            while i < len(insts):
                ins = insts[i]
                si = ins.sync_info
                if si is not None and len(si.on_wait) > max_waits:
                    waits = list(si.on_wait)
                    extra, keep = waits[:-max_waits], waits[-max_waits:]
                    for j in range(0, len(extra), max_waits):
                        nop = mybir.InstNoOp(
                            name=nc.get_next_instruction_name(), ins=[], outs=[])
                        k += 1
                        nop.engine = ins.engine
                        nop.sync_info = mybir.SyncInfo(
                            on_wait=extra[j:j + max_waits], on_update=[])
                        nc.register_instruction(nop, overwrite=True)
                        insts.insert(i, nop)
                        i += 1
                    ins.sync_info = mybir.SyncInfo(
                        on_wait=keep, on_update=list(si.on_update))
                i += 1
    return k


# ---------------------------------------------------------------------------
def _emit(nc, tc, d):
    xT_d, wqk_d, qkb_d, wv_d, vb_d, ostk_d, mask_d, outT_d = d

    with tc.tile_pool(name="persist", bufs=1) as persist:
        xT = persist.tile([128, DC, S], F16)
        wqk = persist.tile([128, DC, 512], F16)
        wv = persist.tile([128, DC, 256], F16)
        qkb = persist.tile([128, 4], F32)
        vb = persist.tile([128, 260], F16)
        ostk = persist.tile([128, 2, DC, 128], F16)
        maskt = persist.tile([128, 2, 896], F16)
        qkT = persist.tile([128, 4, S], F16)
        vaug = persist.tile([128, NSC, HL * 65], F16)
        zstk = persist.tile([128, 2, S], F16)
        ones16 = persist.tile([128, 64], F16)
        nc.vector.memset(ones16[:, :], 1.0)

        # ---- input DMA: spread across the 3 DMA-capable queues (sync=SP,
        # scalar=Act, gpsimd=Pool); earliest-needed pieces first so st=0
        # projections can start ~3us in.
        wqk_r = wqk_d.rearrange("p (c n) -> p c n", c=DC)
        nc.sync.dma_start(out=wqk[:, 0:4, :], in_=wqk_r[:, 0:4, :])
        nc.scalar.dma_start(out=wqk[:, 4:8, :], in_=wqk_r[:, 4:8, :])
        nc.gpsimd.dma_start(out=qkb[:, :], in_=qkb_d[:, :])
        for dc in range(4):
            nc.sync.dma_start(out=xT[:, dc, 0:512],
                              in_=xT_d[dc * 128:(dc + 1) * 128, 0:512])
        for dc in range(4, DC):
            nc.scalar.dma_start(out=xT[:, dc, 0:512],
                                in_=xT_d[dc * 128:(dc + 1) * 128, 0:512])
        nc.gpsimd.dma_start(out=wv[:, :, :], in_=wv_d.rearrange("p (c n) -> p c n", c=DC))
        nc.gpsimd.dma_start(out=vb[:, :], in_=vb_d[:, :])
        nc.gpsimd.dma_start(out=maskt[:, :, :], in_=mask_d.rearrange("p (a n) -> p a n", a=2))
        nc.scalar.dma_start(out=ostk[:, :, :, :], in_=ostk_d.rearrange("p (a d c) -> p a d c", a=2, d=DC))
        for st, eng in ((1, nc.gpsimd), (2, nc.sync), (3, nc.sync)):
            for dc in range(DC):
                eng.dma_start(out=xT[:, dc, st * 512:(st + 1) * 512],
                              in_=xT_d[dc * 128:(dc + 1) * 128, st * 512:(st + 1) * 512])

        with (
            tc.tile_pool(name="psP", bufs=2, space="PSUM") as psP,
            tc.tile_pool(name="psS", bufs=2, space="PSUM") as psS,
            tc.tile_pool(name="psZ", bufs=2, space="PSUM") as psZ,
            tc.tile_pool(name="att", bufs=8) as attp,
            tc.tile_pool(name="nrm", bufs=3) as nrm,
            tc.tile_pool(name="ost", bufs=2) as ostp,
        ):
            def emit_proj_group(st, g):
                """g 0-3: Q/K r-tiles; g 4-7: V 128-chunks of s-tile st."""
                stw = slice(st * 512, (st + 1) * 512)
                if g < 4:
                    r = g
                    ps = psP.tile([128, 512], F32, tag="proj", name=f"qk_{st}_{r}")
                    for dc in range(DC):
                        nc.tensor.matmul(
                            ps,
                            lhsT=wqk[:, dc, r * 128:(r + 1) * 128],
                            rhs=xT[:, dc, stw],
                            start=(dc == 0), stop=(dc == DC - 1),
                        )
                    nc.vector.tensor_scalar_add(
                        out=qkT[:, r, stw], in0=ps, scalar1=qkb[:, r:r + 1])
                else:
                    sc = 4 * st + (g - 4)
                    ps = psP.tile([128, 256], F32, tag="proj", name=f"v_{sc}")
                    for dc in range(DC):
                        nc.tensor.matmul(
                            ps,
                            lhsT=xT[:, dc, sc * 128:(sc + 1) * 128],
                            rhs=wv[:, dc, :],
                            start=(dc == 0), stop=(dc == DC - 1),
                        )
                    vsl = vaug[:, sc, :].rearrange("p (h c) -> p h c", c=65)
                    nc.vector.tensor_copy(vsl[:, :, 0:64],
                                          ps.rearrange("p (h c) -> p h c", c=64))
                    nc.vector.memset(vsl[:, :, 64:65], 1.0)
                    nc.vector.tensor_add(out=vaug[:, sc, :], in0=vaug[:, sc, :], in1=vb)

            def attention(qt, fill):
                stw = slice(qt * 512, (qt + 1) * 512)
                nkc = 4 * (qt + 1)

                def hp_attn(hp):
                    qrt, rt = hp, 2 + hp
                    zaugs = [
                        psZ.tile([65, 512], F32, tag="zaug", name=f"z_{qt}_{2*hp+hi}")
                        for hi in range(2)
                    ]
                    ats = {}

                    def scores(kc):
                        j = kc - 4 * qt  # >=0 on diagonal chunks
                        lo = 128 * j if 0 <= j < 4 else 0
                        sc2 = psS.tile([128, 2, 512], F32, tag="sc",
                                       name=f"sc_{qt}_{hp}_{kc}")
                        for half, p0 in ((0, 0), (1, 64)):
                            nc.tensor.matmul(
                                sc2[:, half, lo:512],
                                lhsT=qkT[p0:p0 + 64, rt, kc * 128:(kc + 1) * 128],
                                rhs=qkT[p0:p0 + 64, qrt, qt * 512 + lo:(qt + 1) * 512],
                                start=True, stop=True,
                                tile_position=(p0, 0),
                            )
                        at = attp.tile([128, 2, 512], F16, tag="at")
                        nc.scalar.activation(out=at[:, :, lo:512], in_=sc2[:, :, lo:512],
                                             func=mybir.ActivationFunctionType.Exp)
                        if 0 <= j < 4:  # causal triangle on the 128-block
                            nc.vector.tensor_mul(
                                out=at[:, :, lo:lo + 128],
                                in0=at[:, :, lo:lo + 128],
                                in1=maskt[:, :, 384:512],
                            )
                        ats[kc] = (at, lo)

                    def av(kc):
                        at, lo = ats.pop(kc)
                        for hi in range(2):
                            nc.tensor.matmul(
                                zaugs[hi][:, lo:512],
                                lhsT=vaug[:, kc, 65 * (2 * hp + hi):65 * (2 * hp + hi) + 65],
                                rhs=at[:, hi, lo:512],
                                start=(kc == 0), stop=(kc == nkc - 1),
                            )

                    scores(0)
                    for kc in range(1, nkc):
                        scores(kc)
                        av(kc - 1)
                        fill()
                    av(nkc - 1)

                    # ---- normalize: z * (1/denom); denom broadcast across
                    # partitions via a K=1 f16 matmul against a ones row.
                    for hi in range(2):
                        zaug = zaugs[hi]
                        rd = nrm.tile([65, 512], F32, tag="rd")
                        nc.vector.reciprocal(rd[64:65, :], zaug[64:65, :])
                        rd16 = nrm.tile([65, 512], F16, tag="rd16")
                        nc.vector.tensor_copy(rd16[64:65, :], rd[64:65, :])
                        rb = psS.tile([64, 512], F32, tag="sc", name=f"rb_{qt}_{hp}_{hi}")
                        nc.tensor.matmul(rb, lhsT=ones16[64:65, :],
                                         rhs=rd16[64:65, :], start=True, stop=True)
                        rdb = nrm.tile([64, 512], F32, tag="rdb")
                        nc.vector.tensor_copy(rdb[:, :], rb)
                        if hi == 0:
                            nc.vector.tensor_mul(out=zstk[0:64, hp, stw],
                                                 in0=zaug[0:64, :], in1=rdb[:, :])
                        else:
                            zs = nrm.tile([64, 512], F16, tag="zs")
                            nc.vector.tensor_mul(out=zs[:, :], in0=zaug[0:64, :],
                                                 in1=rdb[:, :])
                            nc.sync.dma_start(out=zstk[64:128, hp, stw], in_=zs[:, :])

                hp_attn(0)
                hp_attn(1)

                # ---- output projection for this s-tile ----
                for dc in range(DC):
                    po = psZ.tile([128, 512], F32, tag="zaug", name=f"po_{qt}_{dc}")
                    for pair in range(2):
                        nc.tensor.matmul(
                            po,
                            lhsT=ostk[:, pair, dc, :],
                            rhs=zstk[:, pair, stw],
                            start=(pair == 0), stop=(pair == 1),
                        )
                    og = ostp.tile([128, 512], F16, tag="og")
                    nc.vector.tensor_copy(og[:, :], po)
                    nc.gpsimd.dma_start(
                        out=outT_d[dc * 128:(dc + 1) * 128, stw],
                        in_=og[:, :],
                    )

            for g in range(8):
                emit_proj_group(0, g)
            for st in range(NQT):
                fillers = ([lambda g=g: emit_proj_group(st + 1, g) for g in range(8)]
                           if st + 1 < NQT else [])
                slots = [2 * (4 * (st + 1) - 1)]  # fill() calls remaining

                def fill(fillers=fillers, slots=slots):
                    n = (len(fillers) + slots[0] - 1) // max(slots[0], 1)
                    slots[0] -= 1
                    for _ in range(min(n, len(fillers))):
                        fillers.pop(0)()

                attention(st, fill)
                for f in fillers:
                    f()


def build_nc() -> bass.Bass:
    nc = bass.Bass()
    xT_d = nc.dram_tensor("xT", [D, S], F16, kind="ExternalInput")
    wqk_d = nc.dram_tensor("wqk", [128, DC * 512], F16, kind="ExternalInput")
    qkb_d = nc.dram_tensor("qkb", [128, 4], F32, kind="ExternalInput")
    wv_d = nc.dram_tensor("wv", [128, DC * 256], F16, kind="ExternalInput")
    vb_d = nc.dram_tensor("vb", [128, 260], F16, kind="ExternalInput")
    ostk_d = nc.dram_tensor("ostk", [128, 2 * DC * 128], F16, kind="ExternalInput")
    mask_d = nc.dram_tensor("mask", [128, 2 * 896], F16, kind="ExternalInput")
    outT_d = nc.dram_tensor("outT", [D, S], F16, kind="ExternalOutput")

    with tile.TileContext(nc) as tc:
        _emit(nc, tc, (xT_d, wqk_d, qkb_d, wv_d, vb_d, ostk_d, mask_d, outT_d))
    _split_sync_waits(nc)
    return nc


# ---------------------------------------------------------------------------
def _prep_core_inputs(c, x, Qs, Qbs, Ks, Kbs, Vs, Vbs, O):
    b, hg = divmod(c, 4)
    heads = list(range(4 * hg, 4 * hg + 4))
    scale = np.float32(1.0 / np.sqrt(DH))

    xT = np.ascontiguousarray(x[b].T, dtype=np.float16)

    wq = np.concatenate([Qs[h] for h in heads], axis=1)
    wk = np.concatenate([Ks[h] for h in heads], axis=1) * scale
    wqk = np.concatenate([wq, wk], axis=1).astype(np.float16)
    wqk = np.ascontiguousarray(
        wqk.reshape(DC, 128, 512).transpose(1, 0, 2).reshape(128, DC * 512))

    qkb_cols = np.concatenate([Qbs[h] for h in heads] + [Kbs[h] * scale for h in heads])
    qkb = np.ascontiguousarray(qkb_cols.reshape(4, 128).T, dtype=np.float32)

    wv = np.concatenate([Vs[h] for h in heads], axis=1).astype(np.float16)
    wv = np.ascontiguousarray(
        wv.reshape(DC, 128, 256).transpose(1, 0, 2).reshape(128, DC * 256))
    vb = np.zeros((128, 260), dtype=np.float16)
    for hh, h in enumerate(heads):
        vb[:, 65 * hh: 65 * hh + 64] = Vbs[h][None, :]

    o4 = np.stack([O[h] for h in heads])                # [4, 64, 1024]
    ostk = o4.reshape(2, 128, DC, 128).transpose(0, 2, 1, 3).astype(np.float16)
    # [pair, dc, r, c] -> partition-major [r, pair, dc, c] flattened
    ostk = np.ascontiguousarray(
        ostk.transpose(2, 0, 1, 3).reshape(128, 2 * DC * 128))

    t = np.arange(896, dtype=np.int64)[None, :] - 384
    i = np.arange(128, dtype=np.int64)[:, None]
    mask = np.where(t >= i, np.float16(1.0), np.float16(0.0)).astype(np.float16)
    mask2 = np.concatenate([mask, mask], axis=1)

    return {"xT": xT, "wqk": wqk, "qkb": qkb, "wv": wv, "vb": vb,
            "ostk": ostk, "mask": np.ascontiguousarray(mask2)}


def _run(inputs, trace=False, tmpdir=None):
    x = np.asarray(inputs["normalized_resid_pre"], dtype=np.float32)
    Qs = np.asarray(inputs["Qs"], dtype=np.float32)
    Qbs = np.asarray(inputs["Qbs"], dtype=np.float32)
    Ks = np.asarray(inputs["Ks"], dtype=np.float32)
    Kbs = np.asarray(inputs["Kbs"], dtype=np.float32)
    Vs = np.asarray(inputs["Vs"], dtype=np.float32)
    Vbs = np.asarray(inputs["Vbs"], dtype=np.float32)
    O = np.asarray(inputs["O"], dtype=np.float32)
    Ob = np.asarray(inputs["Ob"], dtype=np.float32)

    in_maps = [_prep_core_inputs(c, x, Qs, Qbs, Ks, Kbs, Vs, Vbs, O)
               for c in range(8)]
    last_err = None
    for attempt in range(3):
        try:
            nc = build_nc()
            res = run_bass_kernel_spmd(nc, in_maps, list(range(8)), trace=trace,
                                       tmpdir=tmpdir)
            break
        except Exception as e:  # transient NRT device errors; retry
            last_err = e
    else:
        raise last_err

    out = np.zeros((B, S, D), dtype=np.float32)
    for c in range(8):
        out[c // 4] += res.results[c]["outT"].T.astype(np.float32)
    out += Ob[None, None, :]
    return out, res


def kernel(**inputs) -> np.ndarray:
    out, _ = _run(inputs, trace=False)
    return out
